# revision 66
# baseline (speedup 1.0000x reference)
"""Trainium2 Bass kernel: per-row bincount (BagOfWords) over 8 NeuronCores.

Problem: inputs int32 [16384, 200], values in [0, 1100); output f32
[16384, 1099] = per-row histogram over token ids 1..1099 (bin 0 dropped).

Strategy (pure data parallel): shard the batch over 8 cores (2048 rows
each). Each token id is factorized as v = 32*h + l (h in [0,35),
l in [0,32)) and the per-row histogram becomes a tiny per-row matmul on
the PE systolic array:

    psum[l, h] = sum_j onehot_l(l_j)[l] * onehot_h(h_j)[h]

with the contraction over token slots on the partition dim (k = 128 + 72).
The host stages each 256-row pair k-major as fp16 digit planes
[128 k-slots, [lo(512) | h(512)] row-chunk cols] (a pure input
re-encoding; both digits < 35 are fp16-exact), so a pair enters the chip
as ONE contiguous DMA. One-hot matrices are generated in fp16 with
per-bin is_equal compares split across the Vector (DVE 4x mode), GPSIMD,
and Activation (Abs+Relu pair) engines; since l-bins (0..31) and h-bins
(0..34) share scalar values, one compare op covers both digit planes
([lo | h] adjacent in SBUF) for bins 0..31. Matmul emission is
software-pipelined one 256-row pair behind one-hot generation, with
loads prefetched two pairs ahead. Per-row [32, 35] results are packed
4-across-partitions (PE col-groups) densely per PSUM tile, evicted in
bulk on the Scalar engine as fp16, and DMA'd to a [2048, 1120] output in
(l-major, h) bin order; the host permutes to v = 32h + l, drops bins 0
and 1100+, and concatenates shards. All arithmetic is exact
(integer-valued fp16/f32 counts <= 200).
"""

import numpy as np
from contextlib import ExitStack

import concourse.bass as bass
import concourse.tile as tile
from concourse import bacc, mybir
from concourse.bass_utils import run_bass_kernel_spmd

FP16 = mybir.dt.float16
F32 = mybir.dt.float32
AluOp = mybir.AluOpType
ActFn = mybir.ActivationFunctionType

N_CORES = 8
FULL_B = 16384
S = 200
NL, NH = 32, 35          # v = 32*h + l; psum out = [NL partitions, NH free]
V = NL * NH              # 1120 device bins; host drops 0 and 1100..1119
KA, KB = 128, 72

# engine split for the 35 one-hot compare ops per 256-row pair.
# full bins (l and h share the compare, [128, 1024]): c = 0..31
# half bins (h only, [128, 512]): c = 32, 33, 34
DVE_FULL = tuple(range(0, 25))
POOL_FULL = tuple(range(25, 30))
ACT_FULL = (30, 31)
DVE_HALF = (32,)
POOL_HALF = (34,)
ACT_HALF = (33,)


def _host_consts():
    # activation bias table: col j = -(ACT bin value), last col = +1.0
    act_bins = [float(c) for c in ACT_FULL] + [float(c) for c in ACT_HALF]
    ab = np.zeros((128, len(act_bins) + 1), dtype=np.float32)
    for j, c in enumerate(act_bins):
        ab[:, j] = -c
    ab[:, -1] = 1.0
    return {"actbias": np.ascontiguousarray(ab)}


def _emit_pair_mms(nc, ps_tiles, oh3):
    """Matmuls for one 256-row pair; oh3 = [128, 1024 cols, 35 bins]."""
    for g in range(4):
        ps = ps_tiles[g // 2]              # tile T holds half-pair rows 128T+
        goff = 560 * (g % 2)
        for r in range(64):
            rr = g * 64 + r
            s = r % 4
            q = (r // 4) % 8
            b2 = r // 32
            half = rr // 128
            rloc = rr % 128
            ca = 256 * half + rloc        # chunk A col (k = 0..127)
            cb = ca + 128                  # chunk B col (k = 128..199)
            out_ap = ps[32 * s:32 * s + NL,
                        goff + 280 * b2 + NH * q:goff + 280 * b2 + NH * q + NH]
            nc.tensor.matmul(out_ap,
                             oh3[:, ca, 0:NL],
                             oh3[:, 512 + ca, 0:NH],
                             start=True, stop=False,
                             tile_position=(0, 32 * s))
            nc.tensor.matmul(out_ap,
                             oh3[0:KB, cb, 0:NL],
                             oh3[0:KB, 512 + cb, 0:NH],
                             start=False, stop=True,
                             tile_position=(0, 32 * s))


def _emit_pair_evict(nc, ps_tiles, stage, pair, y, split=False):
    """Stage copies (Act) + output DMAs (SP) for the pair just matmul'd."""
    for t in range(2):
        nc.scalar.copy(stage[:, 1120 * t:1120 * (t + 1)], ps_tiles[t][:])
    E = pair
    if split:
        # tail: per-(s, T) DMAs so T=0 rows ship before T=1 is staged
        for t in range(2):
            for s in range(4):
                src = stage[32 * s:32 * s + NL,
                            1120 * t:1120 * (t + 1)].rearrange(
                    "p (i h) -> p i h", h=NH)
                dst = bass.AP(y, (256 * E + 128 * t + s) * V,
                              [[NH, NL], [4 * V, 32], [1, NH]])
                nc.sync.dma_start(dst, src)
        return
    # 4 output DMAs, one per s: dst row = 256E + s + 4*i' where
    # i' = 32T + 16G + 8b2 + q matches stage col 35*i' + h exactly.
    for s in range(4):
        src = stage[32 * s:32 * s + NL, :].rearrange("p (i h) -> p i h", h=NH)
        dst = bass.AP(y, (256 * E + s) * V, [[NH, NL], [4 * V, 64], [1, NH]])
        nc.sync.dma_start(dst, src)


def _kernel_body(ctx, tc, y, x, actbias_d):
    B = FULL_B // N_CORES
    nc = tc.nc
    NP = B // 256  # pairs

    const_pool = ctx.enter_context(tc.tile_pool(name="const", bufs=1))
    kt_pool = ctx.enter_context(tc.tile_pool(name="kt", bufs=2))
    oh_pool = ctx.enter_context(tc.tile_pool(name="oh", bufs=2))
    scr_pool = ctx.enter_context(tc.tile_pool(name="scr", bufs=2))
    mm_psum = ctx.enter_context(tc.tile_pool(name="mm", bufs=1, space="PSUM"))
    stage_pool = ctx.enter_context(tc.tile_pool(name="stage", bufs=2))

    nab = len(ACT_FULL) + len(ACT_HALF) + 1
    ab = const_pool.tile([128, nab], F32, tag="ab")
    act_bias_col = {}
    for j, c in enumerate(list(ACT_FULL) + list(ACT_HALF)):
        act_bias_col[c] = j
    one_col = nab - 1

    ps_tiles = []
    for i in range(2):
        ps = mm_psum.tile([128, 1120], F32, tag=f"ps{i}")
        ps_tiles.append(ps)

    def load_pair(p):
        # one contiguous DMA: the host staged this pair k-major and
        # digit-split as [128 k-slots, [lo(512) | h(512)] cols] fp16
        dig = kt_pool.tile([128, 1024], FP16, tag="dig")
        nc.sync.dma_start(dig[:], bass.AP(x, p * 128 * 1024,
                                          [[1024, 128], [1, 1024]]))
        return dig

    def compares(dig):
        oh = oh_pool.tile([128, NH * 1024], FP16, tag="oh")
        dig_full = dig[:, 0:1024]
        dig_half = dig[:, 512:1024]
        for c in ACT_FULL:  # first on the Act queue, ahead of evictions
            t1 = scr_pool.tile([128, 1024], FP16, tag="t1")
            nc.scalar.activation(t1[:], dig_full, ActFn.Abs,
                                 bias=ab[:, act_bias_col[c]:act_bias_col[c] + 1])
            nc.scalar.activation(oh[:, 1024 * c:1024 * (c + 1)], t1[:],
                                 ActFn.Relu, bias=ab[:, one_col:one_col + 1],
                                 scale=-1.0)
        for c in ACT_HALF:
            t1 = scr_pool.tile([128, 512], FP16, tag="t1h")
            nc.scalar.activation(t1[:], dig_half, ActFn.Abs,
                                 bias=ab[:, act_bias_col[c]:act_bias_col[c] + 1])
            nc.scalar.activation(oh[:, 1024 * c + 512:1024 * (c + 1)], t1[:],
                                 ActFn.Relu, bias=ab[:, one_col:one_col + 1],
                                 scale=-1.0)
        for c in POOL_FULL:
            nc.gpsimd.tensor_scalar(oh[:, 1024 * c:1024 * (c + 1)],
                                    dig_full, float(c), None, AluOp.is_equal)
        for c in POOL_HALF:
            nc.gpsimd.tensor_scalar(oh[:, 1024 * c + 512:1024 * (c + 1)],
                                    dig_half, float(c), None, AluOp.is_equal)
        for c in DVE_FULL:
            nc.vector.tensor_scalar(oh[:, 1024 * c:1024 * (c + 1)],
                                    dig_full, float(c), None, AluOp.is_equal)
        for c in DVE_HALF:
            nc.vector.tensor_scalar(oh[:, 1024 * c + 512:1024 * (c + 1)],
                                    dig_half, float(c), None, AluOp.is_equal)
        return oh[:].rearrange("p (b c) -> p c b", b=NH)

    # ---- software pipeline: loads 2 ahead, transpose/digits 1 ahead,
    # matmuls + eviction 1 behind the compares.
    dig_bufs = {0: load_pair(0), 1: load_pair(1)}
    nc.scalar.dma_start(ab[:], actbias_d.ap())
    pend = None
    for p in range(NP):
        if p + 2 < NP:
            dig_bufs[p + 2] = load_pair(p + 2)
        # (loads two ahead)
        if pend is not None:
            _emit_pair_mms(nc, ps_tiles, pend[1])
        oh3 = compares(dig_bufs.pop(p))
        if pend is not None:
            _emit_pair_evict(nc, ps_tiles, pend[0], pend[2], y)
        stage = stage_pool.tile([128, 2240], FP16, tag="stage")
        pend = (stage, oh3, p)
    _emit_pair_mms(nc, ps_tiles, pend[1])
    _emit_pair_evict(nc, ps_tiles, pend[0], pend[2], y, split=True)


def _build_program():
    B = FULL_B // N_CORES
    nc = bacc.Bacc("TRN2", target_bir_lowering=False, debug=False,
                   num_devices=N_CORES)
    x = nc.dram_tensor("x", [B // 256 * 128, 1024], FP16, kind="ExternalInput")
    nab = len(ACT_FULL) + len(ACT_HALF) + 1
    actbias = nc.dram_tensor("actbias", [128, nab], F32, kind="ExternalInput")
    y = nc.dram_tensor("y", [B, V], FP16, kind="ExternalOutput")
    with tile.TileContext(nc) as tc:
        with ExitStack() as ctx:
            _kernel_body(ctx, tc, y, x, actbias)
    nc.compile()
    return nc


_program_cache = {}


def _get_program():
    if "nc" not in _program_cache:
        _program_cache["nc"] = _build_program()
    return _program_cache["nc"]


def _stage_kmajor(x16):
    """[B, 200] int16 -> [B//256, 128, [lo | h]] k-major digit planes."""
    npairs = x16.shape[0] // 256
    xs = x16.reshape(npairs, 2, 128, S)
    arr = np.zeros((npairs, 128, 1024), np.float16)
    for base, xv in ((0, xs & 31), (512, xs >> 5)):
        arr[:, :, base + 0:base + 128] = xv[:, 0, :, 0:128].transpose(0, 2, 1)
        arr[:, 0:KB, base + 128:base + 256] = \
            xv[:, 0, :, 128:S].transpose(0, 2, 1)
        arr[:, :, base + 256:base + 384] = xv[:, 1, :, 0:128].transpose(0, 2, 1)
        arr[:, 0:KB, base + 384:base + 512] = \
            xv[:, 1, :, 128:S].transpose(0, 2, 1)
    return arr.reshape(npairs * 128, 1024)


def kernel(**inputs) -> np.ndarray:
    B = FULL_B // N_CORES
    x_full = np.asarray(inputs["inputs"])
    assert x_full.shape == (FULL_B, S), x_full.shape
    x16 = np.ascontiguousarray(x_full.astype(np.int16))

    nc = _get_program()
    consts = _host_consts()
    in_maps = []
    for c in range(N_CORES):
        m = {"x": _stage_kmajor(x16[c * B:(c + 1) * B])}
        m.update(consts)
        in_maps.append(m)

    res = run_bass_kernel_spmd(nc, in_maps, core_ids=list(range(N_CORES)))
    ys = [np.asarray(res.results[c]["y"]) for c in range(N_CORES)]
    full = np.concatenate(ys, axis=0).astype(np.float32)
    # device bin order is (l, h); v = 32*h + l -> permute to v order
    full = full.reshape(FULL_B, NL, NH).transpose(0, 2, 1).reshape(FULL_B, V)
    return np.ascontiguousarray(full[:, 1:1100])


# revision 69
# speedup vs baseline: 1.0355x; 1.0355x over previous
"""Trainium2 Bass kernel: per-row bincount (BagOfWords) over 8 NeuronCores.

Problem: inputs int32 [16384, 200], values in [0, 1100); output f32
[16384, 1099] = per-row histogram over token ids 1..1099 (bin 0 dropped).

Strategy (pure data parallel): shard the batch over 8 cores (2048 rows
each). Each token id is factorized as v = 32*h + l (h in [0,35),
l in [0,32)) and the per-row histogram becomes a tiny per-row matmul on
the PE systolic array:

    psum[l, h] = sum_j onehot_l(l_j)[l] * onehot_h(h_j)[h]

with the contraction over token slots on the partition dim (k = 128 + 72).
The host stages each 256-row pair k-major as fp16 digit planes
[128 k-slots, [lo(512) | h(512)] row-chunk cols] (a pure input
re-encoding; both digits < 35 are fp16-exact), so a pair enters the chip
as ONE contiguous DMA. One-hot matrices are generated in fp16 with
per-bin is_equal compares split across the Vector (DVE 4x mode), GPSIMD,
and Activation (Abs+Relu pair) engines; since l-bins (0..31) and h-bins
(0..34) share scalar values, one compare op covers both digit planes
([lo | h] adjacent in SBUF) for bins 0..31. Matmul emission is
software-pipelined one 256-row pair behind one-hot generation, with
loads prefetched two pairs ahead. Per-row [32, 35] results are packed
4-across-partitions (PE col-groups) densely per PSUM tile, evicted in
bulk on the Scalar engine as fp16, and DMA'd to a [2048, 1120] output in
(l-major, h) bin order; the host permutes to v = 32h + l, drops bins 0
and 1100+, and concatenates shards. All arithmetic is exact
(integer-valued fp16/f32 counts <= 200).
"""

import numpy as np
from contextlib import ExitStack

import concourse.bass as bass
import concourse.tile as tile
from concourse import bacc, mybir
from concourse.bass_utils import run_bass_kernel_spmd

FP16 = mybir.dt.float16
F32 = mybir.dt.float32
AluOp = mybir.AluOpType
ActFn = mybir.ActivationFunctionType

N_CORES = 8
FULL_B = 16384
S = 200
NL, NH = 32, 35          # v = 32*h + l; psum out = [NL partitions, NH free]
V = NL * NH              # 1120 device bins; host drops 0 and 1100..1119
KA, KB = 128, 72

# The host stages shifted digit copies [lo|h|lo-1|h-1|lo-2|h-2|lo-3|h-3],
# so one is_equal against scalar c yields bins c..c+3 (4096-wide quad op),
# c..c+1 (2048-wide double op over the first half) or a single bin.
DVE_QUAD = (0, 4, 8, 12, 16, 20)   # bins 0..23
DVE_DOUBLE = (24,)                 # bins 24,25
POOL_QUAD = (26,)                  # bins 26..29
ACT_DOUBLE = (30,)                 # bins 30,31
POOL_H2 = (32,)                    # h-only bins 32,33 (h, h-1 sections)
ACT_H1 = (34,)                     # h-only bin 34 (h-2 section, scalar 32)


def _host_consts():
    # activation bias table: col 0 = -30 (ACT double), col 1 = -32 (ACT h1),
    # col 2 = +1.0
    ab = np.zeros((128, 3), dtype=np.float32)
    ab[:, 0] = -30.0
    ab[:, 1] = -32.0
    ab[:, 2] = 1.0
    return {"actbias": np.ascontiguousarray(ab)}


def _emit_pair_mms(nc, ps_tiles, oh3):
    """Matmuls for one 256-row pair; oh3 = [128, 1024 cols, 35 bins]."""
    for g in range(4):
        ps = ps_tiles[g // 2]              # tile T holds half-pair rows 128T+
        goff = 560 * (g % 2)
        for r in range(64):
            rr = g * 64 + r
            s = r % 4
            q = (r // 4) % 8
            b2 = r // 32
            half = rr // 128
            rloc = rr % 128
            ca = 256 * half + rloc        # chunk A col (k = 0..127)
            cb = ca + 128                  # chunk B col (k = 128..199)
            out_ap = ps[32 * s:32 * s + NL,
                        goff + 280 * b2 + NH * q:goff + 280 * b2 + NH * q + NH]
            nc.tensor.matmul(out_ap,
                             oh3[:, ca, 0:NL],
                             oh3[:, 512 + ca, 0:NH],
                             start=True, stop=False,
                             tile_position=(0, 32 * s))
            nc.tensor.matmul(out_ap,
                             oh3[0:KB, cb, 0:NL],
                             oh3[0:KB, 512 + cb, 0:NH],
                             start=False, stop=True,
                             tile_position=(0, 32 * s))


def _emit_pair_evict(nc, ps_tiles, stage, pair, y, split=False):
    """Stage copies (Act) + output DMAs (SP) for the pair just matmul'd."""
    for t in range(2):
        nc.scalar.copy(stage[:, 1120 * t:1120 * (t + 1)], ps_tiles[t][:])
    E = pair
    if split:
        # tail: per-(s, T) DMAs so T=0 rows ship before T=1 is staged
        for t in range(2):
            for s in range(4):
                src = stage[32 * s:32 * s + NL,
                            1120 * t:1120 * (t + 1)].rearrange(
                    "p (i h) -> p i h", h=NH)
                dst = bass.AP(y, (256 * E + 128 * t + s) * V,
                              [[NH, NL], [4 * V, 32], [1, NH]])
                nc.sync.dma_start(dst, src)
        return
    # 4 output DMAs, one per s: dst row = 256E + s + 4*i' where
    # i' = 32T + 16G + 8b2 + q matches stage col 35*i' + h exactly.
    for s in range(4):
        src = stage[32 * s:32 * s + NL, :].rearrange("p (i h) -> p i h", h=NH)
        dst = bass.AP(y, (256 * E + s) * V, [[NH, NL], [4 * V, 64], [1, NH]])
        nc.sync.dma_start(dst, src)


def _kernel_body(ctx, tc, y, x, actbias_d):
    B = FULL_B // N_CORES
    nc = tc.nc
    NP = B // 256  # pairs

    const_pool = ctx.enter_context(tc.tile_pool(name="const", bufs=1))
    kt_pool = ctx.enter_context(tc.tile_pool(name="kt", bufs=2))
    oh_pool = ctx.enter_context(tc.tile_pool(name="oh", bufs=2))
    scr_pool = ctx.enter_context(tc.tile_pool(name="scr", bufs=1))
    mm_psum = ctx.enter_context(tc.tile_pool(name="mm", bufs=1, space="PSUM"))
    stage_pool = ctx.enter_context(tc.tile_pool(name="stage", bufs=2))

    ab = const_pool.tile([128, 3], F32, tag="ab")

    ps_tiles = []
    for i in range(2):
        ps = mm_psum.tile([128, 1120], F32, tag=f"ps{i}")
        ps_tiles.append(ps)

    def load_pair(p):
        # one contiguous DMA: the host staged this pair k-major and
        # digit-split as [128 k-slots, [lo(512) | h(512)] cols] fp16
        dig = kt_pool.tile([128, 4096], FP16, tag="dig")
        nc.sync.dma_start(dig[:], bass.AP(x, p * 128 * 4096,
                                          [[4096, 128], [1, 4096]]))
        return dig

    def compares(dig):
        oh = oh_pool.tile([128, NH * 1024], FP16, tag="oh")
        for c in ACT_DOUBLE:  # bins c, c+1 via sections 0:2048
            t1 = scr_pool.tile([128, 2048], FP16, tag="t1")
            nc.scalar.activation(t1[:], dig[:, 0:2048], ActFn.Abs,
                                 bias=ab[:, 0:1])
            nc.scalar.activation(oh[:, 1024 * c:1024 * (c + 2)], t1[:],
                                 ActFn.Relu, bias=ab[:, 2:3], scale=-1.0)
        for c in ACT_H1:  # bin 34 via the h-2 section, |x - 32|
            t1 = scr_pool.tile([128, 512], FP16, tag="t1h")
            nc.scalar.activation(t1[:], dig[:, 2560:3072], ActFn.Abs,
                                 bias=ab[:, 1:2])
            nc.scalar.activation(oh[:, 1024 * c + 512:1024 * (c + 1)], t1[:],
                                 ActFn.Relu, bias=ab[:, 2:3], scale=-1.0)
        for c in POOL_QUAD:  # bins c..c+3
            nc.gpsimd.tensor_scalar(oh[:, 1024 * c:1024 * (c + 4)],
                                    dig[:, 0:4096], float(c), None,
                                    AluOp.is_equal)
        for c in POOL_H2:  # h-only bins c, c+1 via h / h-1 sections
            hin = dig[:].rearrange("p (a b) -> p a b", b=1024)[:, 0:2, 512:1024]
            hout = oh[:, 1024 * c:1024 * (c + 2)].rearrange(
                "p (a b) -> p a b", b=1024)[:, :, 512:1024]
            nc.gpsimd.tensor_scalar(hout, hin, float(c), None, AluOp.is_equal)
        for c in DVE_QUAD:  # bins c..c+3
            nc.vector.tensor_scalar(oh[:, 1024 * c:1024 * (c + 4)],
                                    dig[:, 0:4096], float(c), None,
                                    AluOp.is_equal)
        for c in DVE_DOUBLE:  # bins c, c+1 via sections 0:2048
            nc.vector.tensor_scalar(oh[:, 1024 * c:1024 * (c + 2)],
                                    dig[:, 0:2048], float(c), None,
                                    AluOp.is_equal)
        return oh[:].rearrange("p (b c) -> p c b", b=NH)

    # ---- software pipeline: loads 2 ahead, transpose/digits 1 ahead,
    # matmuls + eviction 1 behind the compares.
    dig_bufs = {0: load_pair(0), 1: load_pair(1)}
    nc.scalar.dma_start(ab[:], actbias_d.ap())
    pend = None
    for p in range(NP):
        if p + 2 < NP:
            dig_bufs[p + 2] = load_pair(p + 2)
        # (loads two ahead)
        if pend is not None:
            _emit_pair_mms(nc, ps_tiles, pend[1])
        oh3 = compares(dig_bufs.pop(p))
        if pend is not None:
            _emit_pair_evict(nc, ps_tiles, pend[0], pend[2], y)
        stage = stage_pool.tile([128, 2240], FP16, tag="stage")
        pend = (stage, oh3, p)
    _emit_pair_mms(nc, ps_tiles, pend[1])
    _emit_pair_evict(nc, ps_tiles, pend[0], pend[2], y, split=True)


def _build_program():
    B = FULL_B // N_CORES
    nc = bacc.Bacc("TRN2", target_bir_lowering=False, debug=False,
                   num_devices=N_CORES)
    x = nc.dram_tensor("x", [B // 256 * 128, 4096], FP16, kind="ExternalInput")
    actbias = nc.dram_tensor("actbias", [128, 3], F32, kind="ExternalInput")
    y = nc.dram_tensor("y", [B, V], FP16, kind="ExternalOutput")
    with tile.TileContext(nc) as tc:
        with ExitStack() as ctx:
            _kernel_body(ctx, tc, y, x, actbias)
    nc.compile()
    return nc


_program_cache = {}


def _get_program():
    if "nc" not in _program_cache:
        _program_cache["nc"] = _build_program()
    return _program_cache["nc"]


def _stage_kmajor(x16):
    """[B, 200] int16 -> k-major shifted digit planes
    [B//256, 128, [lo|h|lo-1|h-1|lo-2|h-2|lo-3|h-3]] (pad stays at -k,
    which never equals a bin scalar, so pad one-hots are all zero)."""
    npairs = x16.shape[0] // 256
    xs = x16.reshape(npairs, 2, 128, S)
    base = np.full((npairs, 128, 1024), -1.0, np.float16)
    for b0, xv in ((0, xs & 31), (512, xs >> 5)):
        base[:, :, b0 + 0:b0 + 128] = xv[:, 0, :, 0:128].transpose(0, 2, 1)
        base[:, 0:KB, b0 + 128:b0 + 256] = \
            xv[:, 0, :, 128:S].transpose(0, 2, 1)
        base[:, :, b0 + 256:b0 + 384] = \
            xv[:, 1, :, 0:128].transpose(0, 2, 1)
        base[:, 0:KB, b0 + 384:b0 + 512] = \
            xv[:, 1, :, 128:S].transpose(0, 2, 1)
    arr = np.concatenate([base - k for k in range(4)], axis=2)
    return arr.reshape(npairs * 128, 4096)


def kernel(**inputs) -> np.ndarray:
    B = FULL_B // N_CORES
    x_full = np.asarray(inputs["inputs"])
    assert x_full.shape == (FULL_B, S), x_full.shape
    x16 = np.ascontiguousarray(x_full.astype(np.int16))

    nc = _get_program()
    consts = _host_consts()
    in_maps = []
    for c in range(N_CORES):
        m = {"x": _stage_kmajor(x16[c * B:(c + 1) * B])}
        m.update(consts)
        in_maps.append(m)

    res = run_bass_kernel_spmd(nc, in_maps, core_ids=list(range(N_CORES)))
    ys = [np.asarray(res.results[c]["y"]) for c in range(N_CORES)]
    full = np.concatenate(ys, axis=0).astype(np.float32)
    # device bin order is (l, h); v = 32*h + l -> permute to v order
    full = full.reshape(FULL_B, NL, NH).transpose(0, 2, 1).reshape(FULL_B, V)
    return np.ascontiguousarray(full[:, 1:1100])
